# revision 1
# baseline (speedup 1.0000x reference)
"""GAT forward kernel for Trainium2 (8 NeuronCores, Bass/Tile).

Reference computation (dense form):
    adj = densify(A); Wh = X @ Ws; e = leaky_relu(Wh@a1 + (Wh@a2).T, 0.15)
    att = softmax(where(adj>0, e, -9e15), axis=1); out = elu(att @ Wh)

This kernel exploits sparsity: only ~524K edges out of 16384^2 matter.
Because |e| <= ~16 (bounded inputs), softmax needs no max-subtraction:
    w_e = exp(leaky(s_src + t_dst));  out_i = elu(sum_e w_e Wh_dst / sum_e w_e)
with exp(-9e15) == 0 handled by simply not summing non-edges, and duplicate
edges deduplicated on the host (reference only uses adj > 0).

Sharding: rows (softmax queries) split 2048/core across 8 cores. Each core:
  P1: computes Wh = X @ Ws (PE transpose + matmul), s = Wh@a1, t = Wh@a2
      for ALL nodes, writes a DRAM table row j = [t_j f32, s_j f32, Wh_j bf16] (256B).
  P3: dma_gathers table rows by edge dst (512B) and the 256B head window by
      edge src (elem_step trick) for its own edges; computes w on DVE/ACT;
      segment-aggregates per 128-row block via one-hot PE matmuls:
          acc[128,65] += onehot(srcrel)[128e,128r].T @ (w * [Wh_dst, 1])
      then out = elu(U / Z) and writes its 2048 output rows.

Host prep packs edges into per-(core,block) buckets padded to a cross-core
uniform chunk count so all 8 cores run the same program (SPMD).
"""
import os
import sys

if "/opt/trn_rl_repo" not in sys.path:
    sys.path.insert(0, "/opt/trn_rl_repo")

_ABL = set(os.environ.get("GAT_ABLATE", "").split(","))

from contextlib import ExitStack

import numpy as np

import concourse.bass as bass
import concourse.tile as tile
from concourse import bacc, mybir
from concourse.bass_utils import run_bass_kernel_spmd
from concourse.masks import make_identity

N = 16384          # nodes
F = 128            # input features
D = 64             # embedding dim
NCORES = 8
R = N // NCORES    # rows per core (2048)
NB = R // 128      # row blocks per core (16)
NBG = N // 128     # global node blocks (128)
TW = 64            # table row width in f32 slots (256 bytes)
dt = mybir.dt


# ---------------------------------------------------------------- host prep
def _prep_edges(A):
    """Dedup edges, bucket by (core, block) with each row's edges padded to a
    multiple of 16 (so every 16-slot "cell" belongs to one src row), then pad
    blocks to cross-core uniform chunk counts Kb. Returns per-core index /
    srcrel / cell arrays and the shared Kb."""
    src_all = np.asarray(A[0], dtype=np.int64)
    dst_all = np.asarray(A[1], dtype=np.int64)
    keys = np.unique(src_all * N + dst_all)     # dedup + sort by (src, dst)
    src = (keys // N).astype(np.int32)
    dst = (keys % N).astype(np.int32)

    deg = np.bincount(src, minlength=N)
    assert deg.min() > 0, (
        "empty rows present; this kernel assumes every row has >=1 edge"
    )
    deg16 = ((deg + 15) // 16) * 16              # 16-aligned row sizes
    gb = np.arange(N) >> 7
    cnt16 = np.bincount(gb, weights=deg16, minlength=NBG).astype(np.int64)
    cnt16 = cnt16.reshape(NCORES, NB)
    Kb = np.maximum((cnt16.max(axis=0) + 127) // 128, 1)          # [NB]
    S = int(Kb.sum()) * 128                      # slots per core
    offs = np.concatenate([[0], np.cumsum(Kb)]) * 128  # slot offset per block
    # cells per block, padded to 128-cell granularity for the gather
    ncell = [int(k) * 8 for k in Kb]
    ncellp = [((n + 127) // 128) * 128 for n in ncell]
    cell_offs = np.concatenate([[0], np.cumsum(ncellp)])
    SC = int(cell_offs[-1])                      # padded cells per core

    row_start = np.concatenate([[0], np.cumsum(deg)])

    dsti = np.zeros((NCORES, S), np.int16)       # table idx for dst gather
    srcrel = np.full((NCORES, S), -1.0, np.float32)  # row-in-block, -1 = pad
    cellsrc = np.zeros((NCORES, SC), np.int16)   # global src row per cell
    for c in range(NCORES):
        for b in range(NB):
            rows = np.arange((c * NB + b) * 128, (c * NB + b) * 128 + 128)
            pos = offs[b]
            for r in rows:
                d = int(deg[r])
                lo = row_start[r]
                dsti[c, pos:pos + d] = dst[lo:lo + d]
                srcrel[c, pos:pos + d] = float(r & 127)
                nc16 = int(deg16[r])
                cbase = cell_offs[b] + (pos - offs[b]) // 16
                cellsrc[c, cbase:cbase + nc16 // 16] = r
                pos += nc16
            assert pos <= offs[b + 1]

    def wrap(x):
        n = x.shape[0]
        w = x.reshape(n // 16, 16).T             # [16, n/16]
        return np.tile(w, (8, 1)).copy()         # [128, n/16]

    cores = []
    for c in range(NCORES):
        cores.append({
            "dsti": wrap(dsti[c]),
            "celli": wrap(cellsrc[c]),
            "srcrel": srcrel[c].reshape(S // 128, 128).T.copy(),  # [128, S/128]
        })
    return cores, [int(k) for k in Kb], S, [int(x) for x in ncellp]


_qctr = [0]


def _q():
    # Strict issue-order rotation over the 4 SWDGE queues: Tile assigns DMA-SW
    # sem lanes round-robin (k % 8), so queue = k % 4 keeps every lane pinned
    # to one queue (8 % 4 == 0).
    return 0


# ---------------------------------------------------------------- device IR
def _build(Kb, S, ncellp):
    _qctr[0] = 0
    SC = sum(ncellp)
    nc = bacc.Bacc("TRN2", target_bir_lowering=False, debug=False,
                   enable_asserts=False, num_devices=NCORES,
                   num_swdge_queues=4)
    X_d = nc.dram_tensor("X", [N, F], dt.float32, kind="ExternalInput").ap()
    Ws_d = nc.dram_tensor("Ws", [F, D], dt.float32, kind="ExternalInput").ap()
    a1_d = nc.dram_tensor("a1b", [128, D], dt.float32, kind="ExternalInput").ap()
    a2_d = nc.dram_tensor("a2b", [128, D], dt.float32, kind="ExternalInput").ap()
    dsti_d = nc.dram_tensor("dsti", [128, S // 16], dt.int16, kind="ExternalInput").ap()
    celli_d = nc.dram_tensor("celli", [128, SC // 16], dt.int16, kind="ExternalInput").ap()
    srel_d = nc.dram_tensor("srcrel", [128, S // 128], dt.float32, kind="ExternalInput").ap()
    sel8_d = nc.dram_tensor("sel8", [128, 16], dt.float32, kind="ExternalInput").ap()
    E8_d = nc.dram_tensor("E8", [128, 128], dt.float32, kind="ExternalInput").ap()
    out_d = nc.dram_tensor("out", [R, D], dt.float32, kind="ExternalOutput").ap()

    with tile.TileContext(nc) as tc, ExitStack() as ctx:
        cpool = ctx.enter_context(tc.tile_pool(name="const", bufs=1))
        dram = ctx.enter_context(tc.tile_pool(name="dram", bufs=1, space="DRAM"))
        xpool = ctx.enter_context(tc.tile_pool(name="x", bufs=2))
        xtpool = ctx.enter_context(tc.tile_pool(name="xt", bufs=2))
        twpool = ctx.enter_context(tc.tile_pool(name="tw", bufs=2))
        tmppool = ctx.enter_context(tc.tile_pool(name="tmp", bufs=2))
        # PSUM budget (8 banks): big pool 2x2 banks shared by P1 transposes
        # and nothing else; small pool 2x1 shared by P1 Wh and P3 acc;
        # s-expand pool 2x1.
        ps_big = ctx.enter_context(tc.tile_pool(name="ps_big", bufs=2, space="PSUM"))
        ps_sm = ctx.enter_context(tc.tile_pool(name="ps_sm", bufs=2, space="PSUM"))
        ps_se = ctx.enter_context(tc.tile_pool(name="ps_se", bufs=2, space="PSUM"))
        gpool = ctx.enter_context(tc.tile_pool(name="gat", bufs=4))
        spool = ctx.enter_context(tc.tile_pool(name="sg", bufs=4))
        wpool = ctx.enter_context(tc.tile_pool(name="w", bufs=2))
        Gpool = ctx.enter_context(tc.tile_pool(name="G", bufs=3))
        ohpool = ctx.enter_context(tc.tile_pool(name="oh", bufs=2))
        epool = ctx.enter_context(tc.tile_pool(name="ep", bufs=2))

        tabTW = dram.tile([N, TW], dt.float32)   # [t, s, Wh(64), garbage pad]

        # ---- constants
        ident = cpool.tile([128, 128], dt.float32)
        make_identity(nc, ident[:])
        iota_i = cpool.tile([128, 128], dt.int32)
        nc.gpsimd.iota(iota_i[:], pattern=[[1, 128]], base=0, channel_multiplier=0)
        iota_f = cpool.tile([128, 128], dt.float32)
        nc.vector.tensor_copy(iota_f[:], iota_i[:])
        ws_t = cpool.tile([F, D], dt.float32)
        nc.sync.dma_start(ws_t[:], Ws_d)
        a1_t = cpool.tile([128, D], dt.float32)
        nc.sync.dma_start(a1_t[:], a1_d)
        a2_t = cpool.tile([128, D], dt.float32)
        nc.sync.dma_start(a2_t[:], a2_d)
        dsti_t = cpool.tile([128, S // 16], dt.int16)
        nc.sync.dma_start(dsti_t[:], dsti_d)
        celli_t = cpool.tile([128, SC // 16], dt.int16)
        nc.sync.dma_start(celli_t[:], celli_d)
        srel_t = cpool.tile([128, S // 128], dt.float32)
        nc.sync.dma_start(srel_t[:], srel_d)
        sel8_t = cpool.tile([128, 16], dt.float32)
        nc.sync.dma_start(sel8_t[:], sel8_d)
        E8_t = cpool.tile([128, 128], dt.float32)
        nc.sync.dma_start(E8_t[:], E8_d)

        # ---- P1: build table row j = [t_j, s_j, Wh_j(64), pad] for all nodes
        X_v = X_d.rearrange("(q k p) f -> q p k f", p=128, k=8)   # [16, 128, 8, F]
        tab_v = tabTW[:].rearrange("(q k p) w -> p q k w", p=128, k=8)
        for q in range(16):                       # groups of 8 node blocks
            xb8 = xpool.tile([128, 8, F], dt.float32)
            nc.scalar.dma_start(xb8[:], X_v[q])
            tw = twpool.tile([128, 8, 34], dt.float32)
            xt8_ps = ps_big.tile([128, 8, 128], dt.float32, space="PSUM", tag="big")
            if "nop1" not in _ABL:
                for k in range(8):
                    nc.tensor.transpose(xt8_ps[:, k, :], xb8[:, k, :], ident[:])
                xt8 = xtpool.tile([128, 8, 128], dt.float32)
                nc.vector.tensor_copy(xt8[:], xt8_ps[:])
                wh_ps8 = ps_sm.tile([128, 8, D], dt.float32, space="PSUM", tag="sm")
                for k in range(8):
                    nc.tensor.matmul(wh_ps8[:, k, :], lhsT=xt8[:, k, :],
                                     rhs=ws_t[:], start=True, stop=True)
                nc.vector.tensor_copy(tw[:, :, 2:34].bitcast(dt.bfloat16), wh_ps8[:])
                tmp = tmppool.tile([128, 8, D], dt.float32)
                nc.vector.tensor_mul(
                    tmp[:], wh_ps8[:],
                    a2_t[:, None, :].to_broadcast([128, 8, D]))
                nc.vector.reduce_sum(tw[:, :, 0:1], tmp[:],
                                     axis=mybir.AxisListType.X)
                tmp2 = tmppool.tile([128, 8, D], dt.float32)
                nc.vector.tensor_mul(
                    tmp2[:], wh_ps8[:],
                    a1_t[:, None, :].to_broadcast([128, 8, D]))
                nc.vector.reduce_sum(tw[:, :, 1:2], tmp2[:],
                                     axis=mybir.AxisListType.X)
            # write rows (q*8+k)*128+p, cols 0:34 (pad cols stay garbage --
            # they are gathered but never read by any compute)
            nc.scalar.dma_start(tab_v[:, q, :, 0:34], tw[:])

        # ---- P3: per-block gather + weight + one-hot aggregate + epilogue
        tab_ap = tabTW[:]                                    # [N, 128] rows
        outstage = cpool.tile([128, NB, D], dt.float32)
        off = 0
        cell_off = 0
        for b in range(NB):
            K = Kb[b]
            n_idx = K * 128
            ncp = ncellp[b]                      # padded cell count (x128)
            nm = ncp // 128                      # 16-chunk spans
            # dma_gather is limited to 1024 indices per call (64 descriptors
            # per SDMA engine, single packet) -- split into 8-chunk sub-calls.
            gat = gpool.tile([128, K, TW], dt.float32)
            if "init" in _ABL:
                nc.vector.memzero(gat[:])
            for c0 in range(0, K, 8):
                nch = min(8, K - c0)
                ni = nch * 128
                o = off + c0 * 128
                if "nogat" not in _ABL:
                    nc.gpsimd.dma_gather(
                        out_ap=gat[:, c0:c0 + nch, :], in_ap=tab_ap,
                        idxs_ap=dsti_t[:, o // 16:(o + ni) // 16],
                        num_idxs=ni, num_idxs_reg=ni, elem_size=TW,
                        queue_num=_q(),
                    )
            # s per cell (one row per 16 aligned slots), then expand to the
            # edge layout via a constant matmul:
            #   s_edge[p, 16m+cl] = cellval[8*cl + p//16, m]
            #                     = sum_q E8[q, p] * (cellval[q, m] * sel8[q, cl])
            cellv = spool.tile([128, nm, 64], dt.float32)
            if "nosg" not in _ABL:
                nc.gpsimd.dma_gather(
                    out_ap=cellv[:], in_ap=tab_ap,
                    idxs_ap=celli_t[:, cell_off // 16:(cell_off + ncp) // 16],
                    num_idxs=ncp, num_idxs_reg=ncp, elem_size=TW,
                    queue_num=_q(),
                )
            else:
                nc.vector.memzero(cellv[:])
            s_ps = ps_se.tile([128, nm * 16], dt.float32, space="PSUM", tag="se")
            for m in range(nm):
                rhsm = wpool.tile([128, 16], dt.float32, tag="rhsm")
                nc.vector.tensor_mul(
                    rhsm[:], sel8_t[:],
                    cellv[:, m, 1:2].to_broadcast([128, 16]))
                nc.tensor.matmul(s_ps[:, m * 16:(m + 1) * 16], lhsT=E8_t[:],
                                 rhs=rhsm[:], start=True, stop=True)
            # w = exp(leaky(s + t))
            e_t = wpool.tile([128, K], dt.float32, tag="e")
            nc.vector.tensor_add(e_t[:], s_ps[:, 0:K], gat[:, :, 0])
            lk = wpool.tile([128, K], dt.float32, tag="lk")
            nc.vector.scalar_tensor_tensor(
                out=lk[:], in0=e_t[:], scalar=0.15, op0=mybir.AluOpType.mult,
                in1=e_t[:], op1=mybir.AluOpType.max)
            w_t = wpool.tile([128, K], dt.float32, tag="wt")
            nc.scalar.activation(w_t[:], lk[:], mybir.ActivationFunctionType.Exp)
            # G = [w * Wh_dst, w]
            G = Gpool.tile([128, K, D + 1], dt.float32)
            nc.vector.tensor_mul(G[:, :, 0:D], gat[:, :, 2:34].bitcast(dt.bfloat16),
                                 w_t[:, :, None].to_broadcast([128, K, D]))
            nc.vector.tensor_copy(G[:, :, D], w_t[:])
            # one-hot of srcrel vs row-in-block
            oh = ohpool.tile([128, K, 128], dt.float32)
            if "nooh" in _ABL:
                nc.vector.memzero(oh[:, 0, :])
            else:
                nc.vector.tensor_tensor(
                    out=oh[:],
                    in0=iota_f[:, None, :].to_broadcast([128, K, 128]),
                    in1=srel_t[:, off // 128:off // 128 + K, None]
                        .to_broadcast([128, K, 128]),
                    op=mybir.AluOpType.is_equal)
            # aggregate
            acc = ps_sm.tile([128, D + 1], dt.float32, space="PSUM", tag="sm")
            nmm = 1 if "nomm" in _ABL else K
            for c in range(nmm):
                nc.tensor.matmul(acc[:], lhsT=oh[:, c, :], rhs=G[:, c, :],
                                 start=(c == 0), stop=(c == nmm - 1))
            # epilogue: out = elu(U / Z)
            zg = epool.tile([128, 1], dt.float32, tag="zg")
            nc.vector.tensor_scalar_max(zg[:], acc[:, D:D + 1], 1e-30)
            zr = epool.tile([128, 1], dt.float32, tag="zr")
            nc.vector.reciprocal(zr[:], zg[:])
            x = epool.tile([128, D], dt.float32, tag="x")
            nc.vector.tensor_scalar_mul(x[:], acc[:, 0:D], zr[:])
            mn = epool.tile([128, D], dt.float32, tag="mn")
            nc.vector.tensor_scalar_min(mn[:], x[:], 0.0)
            em = epool.tile([128, D], dt.float32, tag="em")
            nc.scalar.activation(em[:], mn[:], mybir.ActivationFunctionType.Exp)
            rl = epool.tile([128, D], dt.float32, tag="rl")
            nc.vector.tensor_scalar_max(rl[:], x[:], 0.0)
            nc.vector.scalar_tensor_tensor(
                out=outstage[:, b, :], in0=em[:], scalar=-1.0,
                op0=mybir.AluOpType.add, in1=rl[:], op1=mybir.AluOpType.add)
            off += n_idx
            cell_off += ncp

        out_v = out_d.rearrange("(b p) d -> p b d", p=128)   # [128, NB, D]
        nc.sync.dma_start(out_v, outstage[:])
    nc.compile()
    return nc


_cache = {}


def _get_program(Kb, S, ncellp):
    key = (tuple(Kb), S, tuple(ncellp), tuple(sorted(_ABL)))
    if key not in _cache:
        _cache[key] = _build(Kb, S, ncellp)
    return _cache[key]


def make_in_maps(A, X, Ws, a):
    """Host-side sharding: returns (nc, in_maps)."""
    X = np.ascontiguousarray(np.asarray(X, dtype=np.float32))
    Ws = np.ascontiguousarray(np.asarray(Ws, dtype=np.float32))
    a = np.asarray(a, dtype=np.float32).reshape(2 * D)
    a1b = np.tile(a[:D][None, :], (128, 1)).astype(np.float32)
    a2b = np.tile(a[D:][None, :], (128, 1)).astype(np.float32)
    q = np.arange(128)
    sel8 = (q[:, None] // 8 == np.arange(16)[None, :]).astype(np.float32)
    E8 = (q[:, None] % 8 == q[None, :] // 16).astype(np.float32)
    cores, Kb, S, ncellp = _prep_edges(A)
    nc = _get_program(Kb, S, ncellp)
    in_maps = [
        {"X": X, "Ws": Ws, "a1b": a1b, "a2b": a2b, "sel8": sel8, "E8": E8,
         "dsti": c["dsti"], "celli": c["celli"], "srcrel": c["srcrel"]}
        for c in cores
    ]
    return nc, in_maps


def kernel(A, X, Ws, a):
    nc, in_maps = make_in_maps(A, X, Ws, a)
    res = run_bass_kernel_spmd(nc, in_maps, core_ids=list(range(NCORES)),
                               trace=False)
    return np.concatenate([r["out"] for r in res.results], axis=0)

